# revision 2
# baseline (speedup 1.0000x reference)
"""Causal self-attention (B=2, T=2048, C=1024, H=16, d=64) on 8 Trainium2 NeuronCores.

Strategy (tensor-parallel over heads, two SPMD launches), fp8-DoubleRow edition:
  All heavy matmuls run as fp8e4 (e4m3) DoubleRow pairs (0.5 PE cycles per
  output column, 256-deep contraction per instruction).  e4m3 has a 2^-6
  minimum normal, so every tensor is pre-scaled by a power of two on the
  host to sit in the normal range, and the scales are folded into the exp
  activation scale / softmax-denominator ones-columns / final host unscale.
  Precision-critical paths (v, ctx, Wo) use hi+lo residual splits (effective
  ~11 mantissa bits, better than fp16); q/k tolerate single fp8 (3.6%).
  Measured end-to-end rel err of this scheme vs the fp32 reference: ~8e-3
  (gate is 2e-2).

  Launch 1 (head-parallel): core c owns heads {2c, 2c+1} (128 proj dims).
    Projections from host-split x8/xl and weight hi/lo tensors:
      q64 = (Wq64+Wql)^T x8 + Wq4^T xl      (12 DoubleRow matmuls / chunk)
      k64 = (Wk64+Wkl)^T x8                 (8)
      v64 = (Wv64+Wvl)^T x8 + Wv4^T xl      (12)
    Scores per (batch, head): DoubleRow with lhsT = (khi, klo) residual pair
    and rhs = (q8, q8): sc = (khi+klo)^T q8 at scale 64*64; exp applies
    0.125/4096.  Causal masking and the folded softmax denominator (ones
    columns, value 64.0 so the 64-scaled v cancels) are as before; AV runs
    in fp16.  Output ctxT [128, 4096] fp16, unscaled.
  Launch 2 (token-parallel, fp8 when bo==0): out*256 = (Wo64+Wol)^T chi
    + Wo64^T clo4 with chi = fp8(4 ctx), clo4 = fp8(4(ctx - chi/4)); host
    divides by 256.  Falls back to the fp16 kernel when bo != 0 (bias row).
"""
import sys

for _p in ("/opt/trn_rl_repo", "/root/.axon_site/_ro/trn_rl_repo"):
    if _p not in sys.path:
        sys.path.insert(0, _p)

import ml_dtypes
import numpy as np

import concourse.bass as bass  # noqa: F401  (registers bass types)
import concourse.tile as tile
from concourse import bacc, mybir
from concourse import bass_utils

B, T, C = 2, 2048, 1024
H, D = 16, 64
NC = 8
BT = B * T                       # 4096 tokens
HPC = H // NC                    # 2 heads per core
PD = HPC * D                     # 128 projection dims per core
P = 128
KS = C // P                      # 8 contraction subtiles
CHUNK = 512                      # token/query chunk
QCH = T // CHUNK                 # 4 query chunks per batch
TPC = CHUNK // P                 # 4 key tiles per chunk
KT = T // P                      # 16 key tiles per batch
CA = C + P                       # 1152 augmented contraction for fp16 phase 2
ROWS2 = BT // NC                 # 512 tokens per core in phase 2

F32 = mybir.dt.float32
F16 = mybir.dt.float16
F8 = mybir.dt.float8e4
E4 = ml_dtypes.float8_e4m3
EXP = mybir.ActivationFunctionType.Exp
DR = mybir.MatmulPerfMode.DoubleRow


def _build_phase1():
    nc = bacc.Bacc("TRN2", target_bir_lowering=False, debug=False, num_devices=NC)
    x8_ap = nc.dram_tensor("x8", [C, BT], F8, kind="ExternalInput").ap()
    xl_ap = nc.dram_tensor("xl", [C, BT], F8, kind="ExternalInput").ap()
    WNAMES = ("wq64", "wql", "wq4", "wk64", "wkl", "wv64", "wvl", "wv4")
    WIDX = {n: i for i, n in enumerate(WNAMES)}
    # all weights in one pre-tiled tensor [P, 8, KS, PD] (name-major so
    # group slices stay contiguous -- small runs pay a 2x DMA penalty)
    wa_ap = nc.dram_tensor("wall", [P, len(WNAMES), KS, PD], F8,
                           kind="ExternalInput").ap()
    t2_ap = nc.dram_tensor("tri2", [P, 2, P], F16, kind="ExternalInput").ap()
    # chunks (0,0) and (1,0) projections are host-peeled (prologue peeling)
    # so attention starts immediately while weights/x stream in
    peel_aps = {}
    for i in range(2):
        peel_aps[f"qp{i}"] = nc.dram_tensor(
            f"qp{i}", [P, CHUNK], F8, kind="ExternalInput").ap()
        peel_aps[f"kp{i}"] = nc.dram_tensor(
            f"kp{i}", [P, 2, CHUNK], F8, kind="ExternalInput").ap()
        peel_aps[f"v{i}"] = nc.dram_tensor(
            f"v{i}", [P, TPC, HPC, 2 * D], F16, kind="ExternalInput").ap()
    ct_ap = nc.dram_tensor("ctxt", [PD, BT], F16, kind="ExternalOutput").ap()

    x8_r = x8_ap.rearrange("(ks p) t -> p ks t", p=P)
    xl_r = xl_ap.rearrange("(ks p) t -> p ks t", p=P)

    with tile.TileContext(nc) as tc:
        with (
            tc.tile_pool(name="const", bufs=1) as const,
            tc.tile_pool(name="qkv", bufs=1) as qkv,
            tc.tile_pool(name="xt", bufs=3) as xtp,
            tc.tile_pool(name="vt", bufs=3) as vtp,
            tc.tile_pool(name="ep", bufs=16) as ep,
            tc.tile_pool(name="outp", bufs=3) as outp,
            tc.tile_pool(name="smallp", bufs=3) as smallp,
            tc.tile_pool(name="pp", bufs=2, space="PSUM") as pp,
            tc.tile_pool(name="scp", bufs=2, space="PSUM") as scp,
            tc.tile_pool(name="ctxp", bufs=2, space="PSUM") as ctxp,
        ):
            w_all = const.tile([P, len(WNAMES), KS, PD], F8, tag="wall")
            t2_sb = const.tile([P, 2, P], F16, tag="tri2")
            tri_sb = t2_sb[:, 0]
            id_sb = t2_sb[:, 1]

            # fp8 q8 tiles (broadcast to both DoubleRow slots via a stride-0
            # AP) and (khi,klo) residual pair tiles, per (batch, chunk)
            qpt = [[qkv.tile([P, CHUNK], F8, tag=f"qp{b}_{cc}", name=f"qp{b}_{cc}")
                    for cc in range(QCH)] for b in range(B)]
            kpt = [[qkv.tile([P, 2, CHUNK], F8, tag=f"kp{b}_{cc}", name=f"kp{b}_{cc}")
                    for cc in range(QCH)] for b in range(B)]
            # v in [token, dim] fp16 layout (64-scaled); columns D..2D are 64.0
            # so the AV matmul emits the 64-scaled softmax denominator and the
            # reciprocal-mul yields unscaled ctx.
            v_sb = [[qkv.tile([P, TPC, HPC, 2 * D], F16, tag=f"v{b}_{cc}",
                              name=f"v{b}_{cc}")
                     for cc in range(QCH)] for b in range(B)]

            def proj_bundles(b, cc):
                """(dma_thunk, [compute thunks]) for one chunk's projections.
                The dma thunk is emitted ~2 attention blocks ahead of use;
                compute thunks are interleaved into the previous chunk's
                attention block so the PE stream has independent work while
                ACT runs exps."""
                gsl = bass.ds(b * T + cc * CHUNK, CHUNK)
                st = {}

                def th_dma():
                    st["x8"] = xtp.tile([P, KS, CHUNK], F8, name="x8_t")
                    nc.sync.dma_start(st["x8"][:], x8_r[:, :, gsl])
                    st["xl"] = xtp.tile([P, KS, CHUNK], F8, name="xl_t")
                    nc.sync.dma_start(st["xl"][:], xl_r[:, :, gsl])

                def mk_chain(ps_name, terms):
                    # returns thunks each emitting `group` DR matmuls
                    mms = []
                    for wn, rk in terms:
                        for t in range(KS // 2):
                            mms.append((wn, rk, slice(2 * t, 2 * t + 2)))
                    n = len(mms)

                    def piece(lo, hi):
                        def th():
                            if lo == 0:
                                st[ps_name] = pp.tile(
                                    [P, CHUNK], F32, tag="pp", name=ps_name
                                )
                            ps = st[ps_name]
                            for i in range(lo, hi):
                                wn, rk, sl = mms[i]
                                nc.tensor.matmul(
                                    ps[:], w_all[:, WIDX[wn], sl, :],
                                    st[rk][:, sl],
                                    start=(i == 0), stop=(i == n - 1),
                                    perf_mode=DR,
                                )
                        return th

                    half = (n + 1) // 2
                    return [piece(0, half), piece(half, n)]

                def th_qcopy():
                    nc.vector.tensor_copy(qpt[b][cc][:], st["ps_q"][:])

                def th_kcopy():
                    kp = kpt[b][cc]
                    nc.vector.tensor_copy(kp[:, 0], st["ps_k"][:])
                    nc.vector.tensor_sub(kp[:, 1], st["ps_k"][:], kp[:, 0])

                def th_vcopy():
                    st["vt"] = vtp.tile([P, CHUNK], F16, name="vt_t")
                    nc.vector.tensor_copy(st["vt"][:], st["ps_v"][:])

                def th_vtrans():
                    st["tr"] = pp.tile([P, CHUNK], F16, tag="pp", name="tr")
                    for j in range(TPC):
                        nc.tensor.transpose(
                            st["tr"][:, bass.ts(j, P)],
                            st["vt"][:, bass.ts(j, P)], id_sb[:],
                        )

                def th_vsb():
                    nc.vector.tensor_copy(
                        v_sb[b][cc][:, :, :, 0:D],
                        st["tr"][:].rearrange("p (j h d) -> p j h d", j=TPC, h=HPC),
                    )

                # q is 2-term (W-residual only): the requantization to fp8
                # for scores dominates its error anyway, and skipping the
                # xl chain unhooks q from the late-arriving xl DMA
                q1, q2 = mk_chain("ps_q", [("wq64", "x8"), ("wql", "x8")])
                k1, k2 = mk_chain("ps_k", [("wk64", "x8"), ("wkl", "x8")])
                v1, v2 = mk_chain("ps_v", [("wv64", "x8"), ("wvl", "x8"), ("wv4", "xl")])
                return th_dma, [k1, k2, th_kcopy, q1, q2, th_qcopy,
                                v1, v2, th_vcopy, th_vtrans, th_vsb]

            LOOK = 1  # kt-unit lookahead: sc/exp of kt+1 emitted before AV of kt

            def att_interleaved(b, ci, bundles, final=False):
                q0 = ci * CHUNK
                nkt = q0 // P + TPC
                nu = nkt
                ctx = [ctxp.tile([2 * D, CHUNK], F32, tag="ctx", name=f"ctx{ci}_{h}")
                       for h in range(HPC)]
                ets = {}

                def emit_sc(kt):
                    j = kt - q0 // P
                    c0 = 0 if j < 0 else P * j
                    kb = slice((kt % TPC) * P, (kt % TPC + 1) * P)
                    sc = scp.tile([P, 2, CHUNK], F32, tag="sc", name="sc")
                    for h in range(HPC):
                        dsl = slice(D * h, D * (h + 1))
                        qb = qpt[b][ci][dsl, c0:].rearrange(
                            "p (o n) -> p o n", o=1
                        ).broadcast_to([D, 2, CHUNK - c0])
                        nc.tensor.matmul(
                            sc[:, h, c0:],
                            kpt[b][kt // TPC][dsl, :, kb],
                            qb,
                            start=True, stop=True, perf_mode=DR,
                        )
                    e_t = ep.tile([P, 2, CHUNK], F16, name="e_t")
                    # one exp over both heads' live columns: halves the
                    # per-instruction ACT access-latency overhead
                    nc.scalar.activation(
                        e_t[:, :, c0:], sc[:, :, c0:], EXP, scale=0.125 / 4096.0
                    )
                    if j >= 0:
                        # early units' masks go to Pool so the DVE stream's
                        # head carries no exp-dependent waits (the next
                        # chunk's PSUM->SBUF copies must flow through first)
                        eng = nc.gpsimd if kt < 3 else nc.vector
                        for h in range(HPC):
                            eng.tensor_mul(
                                e_t[:, h, c0 : c0 + P], e_t[:, h, c0 : c0 + P],
                                tri_sb[:],
                            )
                    ets[kt] = (e_t, c0)

                def emit_av(kt):
                    e_t, c0 = ets.pop(kt)
                    for h in range(HPC):
                        nc.tensor.matmul(
                            ctx[h][:, c0:],
                            v_sb[b][kt // TPC][:, kt % TPC, h, 0 : 2 * D],
                            e_t[:, h, c0:],
                            start=(kt == 0), stop=(kt == nkt - 1),
                        )

                # bundles start after the first few sc units (so a bundle
                # stalled on a DMA never blocks this block's score stream)
                # and spread over a few units so copies still land early
                nb = len(bundles)
                total = nu + LOOK
                start = min(nu, 4)
                span = max(1, min(nu - 1, 9) - start)
                sched = {}
                for i in range(nb):
                    pos = min(total - 1, start + i * span // max(nb - 1, 1))
                    sched.setdefault(pos, []).append(bundles[i])
                for u in range(total):
                    if u < nu:
                        emit_sc(u)
                    for th in sched.get(u, ()):
                        th()
                    if u >= LOOK:
                        emit_av(u - LOOK)
                o_t = outp.tile([PD, CHUNK], F16, name="o_t")
                for h in range(HPC):
                    # (a single PSUM/PSUM divide is illegal: TensorTensor may
                    # read only one non-scalar input from PSUM)
                    r_t = smallp.tile([D, CHUNK], F32, tag="r", name="r_t")
                    nc.vector.reciprocal(r_t[:], ctx[h][D : 2 * D, :])
                    nc.vector.tensor_mul(
                        o_t[D * h : D * (h + 1), :], ctx[h][0:D, :], r_t[:]
                    )
                    if final:
                        # last block: DMA each head's half as soon as its
                        # reciprocal-mul lands, overlapping the other head's
                        nc.sync.dma_start(
                            ct_ap[D * h : D * (h + 1),
                                  b * T + q0 : b * T + q0 + CHUNK],
                            o_t[D * h : D * (h + 1), :],
                        )
                if not final:
                    # mid-stream output DMAs ride the Pool SWDGE queue so
                    # their semaphore waits never stall the SP SEQ (which
                    # must keep issuing x prefetches)
                    nc.gpsimd.dma_start(
                        ct_ap[:, b * T + q0 : b * T + q0 + CHUNK], o_t[:]
                    )

            steps = [(b, cc) for cc in range(QCH) for b in range(B)]
            dmas, comps = {}, {}
            for s in steps:
                if s[1] != 0:
                    dmas[s], comps[s] = proj_bundles(*s)
            # first-use-order loads on the sync queue: qp/kp of the two
            # peeled chunks unblock the first score matmuls, then tri/ident,
            # v0/v1 (first AVs), q/k weights, v weights.  (Not the scalar
            # queue: dma_starts there would stall the ACT SEQ and hence the
            # exp stream behind them.)
            nc.sync.dma_start(qpt[0][0][:], peel_aps["qp0"][:])
            nc.sync.dma_start(kpt[0][0][:], peel_aps["kp0"][:])
            nc.sync.dma_start(qpt[1][0][:], peel_aps["qp1"][:])
            nc.sync.dma_start(kpt[1][0][:], peel_aps["kp1"][:])
            nc.sync.dma_start(t2_sb[:], t2_ap[:])
            nc.sync.dma_start(v_sb[0][0][:], peel_aps["v0"][:])
            nc.sync.dma_start(v_sb[1][0][:], peel_aps["v1"][:])
            nc.sync.dma_start(w_all[:, 3:5], wa_ap[:, 3:5])
            dmas.pop(steps[2])()
            nc.sync.dma_start(w_all[:, 0:2], wa_ap[:, 0:2])
            nc.sync.dma_start(w_all[:, 5:], wa_ap[:, 5:])
            for i, (b, cc) in enumerate((b, cc) for b in range(B) for cc in range(QCH)):
                if cc == 0:
                    continue
                eng = nc.vector if i % 2 == 0 else nc.gpsimd
                eng.memset(v_sb[b][cc][:, :, :, D : 2 * D], 64.0)
            for j in range(1, len(steps) + 1):
                pre = dmas.pop(steps[j + 1], None) if j + 1 < len(steps) else None
                if pre is not None:
                    pre()
                att_interleaved(*steps[j - 1],
                                comps.pop(steps[j], []) if j < len(steps) else [],
                                final=(j == len(steps)))

    nc.compile()
    return nc


def _build_phase2_fp8():
    MT = ROWS2 // P              # 4 token tiles
    NT = C // CHUNK              # 2 output column tiles
    nc = bacc.Bacc("TRN2", target_bir_lowering=False, debug=False, num_devices=NC)
    chi_ap = nc.dram_tensor("chi", [C, ROWS2], F8, kind="ExternalInput").ap()
    clo_ap = nc.dram_tensor("clo", [C, ROWS2], F8, kind="ExternalInput").ap()
    wo64_ap = nc.dram_tensor("wo64", [C, C], F8, kind="ExternalInput").ap()
    wol_ap = nc.dram_tensor("wol", [C, C], F8, kind="ExternalInput").ap()
    o_ap = nc.dram_tensor("o", [ROWS2, C], F16, kind="ExternalOutput").ap()

    chi_r = chi_ap.rearrange("(ks p) t -> p ks t", p=P)
    clo_r = clo_ap.rearrange("(ks p) t -> p ks t", p=P)
    wo64_r = wo64_ap.rearrange("(ks p) n -> p ks n", p=P)
    wol_r = wol_ap.rearrange("(ks p) n -> p ks n", p=P)

    with tile.TileContext(nc) as tc:
        with (
            tc.tile_pool(name="ctp", bufs=8) as ctp,
            tc.tile_pool(name="wop", bufs=3) as wop,
            tc.tile_pool(name="outp", bufs=4) as outp,
            tc.tile_pool(name="ps", bufs=1, space="PSUM") as psp,
        ):
            NKP = KS // 2
            # n-outer: finish output-column half n before n+1, so the first
            # half's output copies/DMAs overlap the second half's compute.
            # chi/clo load once in halves on the sync HWDGE queue; the wo
            # tensors ride the Pool SWDGE queue (idle here) because every
            # dma_start costs 625ns on the single shared HWDGE unit.
            ps = [
                [psp.tile([P, CHUNK], F32, tag=f"ps{m}{n}", name=f"ps{m}{n}")
                 for m in range(MT)]
                for n in range(NT)
            ]
            warm = outp.tile([P, CHUNK], F16, name="warm")
            nc.gpsimd.memset(warm[:], 0.0)
            for _ in range(7):
                nc.tensor.matmul(ps[0][0][:], warm[:, 0:P], warm[:],
                                 start=True, stop=True)
            chi_t = ctp.tile([P, KS, ROWS2], F8, name="chi_t")
            clo_t = ctp.tile([P, KS, ROWS2], F8, name="clo_t")
            wo_tiles = {}
            for n in range(NT):
                nb = slice(n * CHUNK, (n + 1) * CHUNK)
                wo64_t = wop.tile([P, KS, CHUNK], F8, name="wo64_t")
                wol_t = wop.tile([P, KS, CHUNK], F8, name="wol_t")
                wo_tiles[n] = (wo64_t, wol_t)
                if n == 0:
                    nc.sync.dma_start(chi_t[:, 0:4], chi_r[:, 0:4])
                    nc.gpsimd.dma_start(wo64_t[:, 0:4], wo64_r[:, 0:4, nb])
                    nc.gpsimd.dma_start(wol_t[:, 0:4], wol_r[:, 0:4, nb])
                    nc.sync.dma_start(chi_t[:, 4:], chi_r[:, 4:])
                    nc.sync.dma_start(clo_t[:, 0:4], clo_r[:, 0:4])
                    nc.gpsimd.dma_start(wo64_t[:, 4:], wo64_r[:, 4:, nb])
                    nc.gpsimd.dma_start(wol_t[:, 4:], wol_r[:, 4:, nb])
                    nc.sync.dma_start(clo_t[:, 4:], clo_r[:, 4:])
                else:
                    nc.gpsimd.dma_start(wo64_t[:], wo64_r[:, :, nb])
                    nc.gpsimd.dma_start(wol_t[:], wol_r[:, :, nb])
            for n in range(NT):
                nb = slice(n * CHUNK, (n + 1) * CHUNK)
                wo64_t, wol_t = wo_tiles[n]
                for t in range(NKP):
                    sl = slice(2 * t, 2 * t + 2)
                    for m in range(MT):
                        mb = slice(m * P, (m + 1) * P)
                        nc.tensor.matmul(
                            ps[n][m][:], chi_t[:, sl, mb], wo64_t[:, sl],
                            start=(t == 0), stop=False, perf_mode=DR,
                        )
                        nc.tensor.matmul(
                            ps[n][m][:], chi_t[:, sl, mb], wol_t[:, sl],
                            start=False, stop=False, perf_mode=DR,
                        )
                        nc.tensor.matmul(
                            ps[n][m][:], clo_t[:, sl, mb], wo64_t[:, sl],
                            start=False, stop=(t == NKP - 1), perf_mode=DR,
                        )
                        if t == NKP - 1:
                            # drain this (m, n) while others still accumulate
                            o_sb = outp.tile([P, CHUNK], F16, name="o_sb")
                            if m % 2 == 0:
                                nc.vector.tensor_copy(o_sb[:], ps[n][m][:])
                            else:
                                nc.scalar.copy(o_sb[:], ps[n][m][:])
                            deng = nc.sync if m % 2 == 0 else nc.scalar
                            deng.dma_start(o_ap[bass.ts(m, P), nb], o_sb[:])

    nc.compile()
    return nc


def _build_phase2_fp16(ca):
    """fp16 fallback, used only when bo != 0 (bias-augmented contraction)."""
    KS2 = ca // P
    MT = ROWS2 // P
    NT = C // CHUNK
    nc = bacc.Bacc("TRN2", target_bir_lowering=False, debug=False, num_devices=NC)
    ct_ap = nc.dram_tensor("cta", [ca, ROWS2], F16, kind="ExternalInput").ap()
    wo_ap = nc.dram_tensor("woa", [ca, C], F16, kind="ExternalInput").ap()
    o_ap = nc.dram_tensor("o", [ROWS2, C], F16, kind="ExternalOutput").ap()

    ct_r = ct_ap.rearrange("(ks p) t -> p ks t", p=P)
    wo_r = wo_ap.rearrange("(ks p) n -> p ks n", p=P)

    with tile.TileContext(nc) as tc:
        with (
            tc.tile_pool(name="ctp", bufs=4) as ctp,
            tc.tile_pool(name="wop", bufs=4) as wop,
            tc.tile_pool(name="outp", bufs=4) as outp,
            tc.tile_pool(name="ps", bufs=1, space="PSUM") as psp,
        ):
            ps = [
                [psp.tile([P, CHUNK], F32, tag=f"ps{m}{n}", name=f"ps{m}{n}")
                 for n in range(NT)]
                for m in range(MT)
            ]
            for k in range(KS2):
                ct_t = ctp.tile([P, ROWS2], F16, name="ct_t")
                nc.sync.dma_start(ct_t[:], ct_r[:, k])
                wo_t = wop.tile([P, C], F16, name="wo_t")
                if k == 0:
                    nc.sync.dma_start(wo_t[:, 0:CHUNK], wo_r[:, k, 0:CHUNK])
                    nc.sync.dma_start(wo_t[:, CHUNK:], wo_r[:, k, CHUNK:])
                else:
                    nc.sync.dma_start(wo_t[:], wo_r[:, k])
                for m in range(MT):
                    for n in range(NT):
                        nc.tensor.matmul(
                            ps[m][n][:],
                            ct_t[:, bass.ts(m, P)],
                            wo_t[:, bass.ts(n, CHUNK)],
                            start=(k == 0), stop=(k == KS2 - 1),
                        )
            for m in range(MT):
                o_sb = outp.tile([P, C], F16, name="o_sb")
                nc.vector.tensor_copy(o_sb[:, 0:CHUNK], ps[m][0][:])
                nc.scalar.copy(o_sb[:, CHUNK:], ps[m][1][:])
                nc.sync.dma_start(o_ap[bass.ts(m, P), :], o_sb[:])

    nc.compile()
    return nc


_CACHE = {}


def _phase1():
    if "p1" not in _CACHE:
        _CACHE["p1"] = _build_phase1()
    return _CACHE["p1"]


def _phase2():
    if "p2" not in _CACHE:
        _CACHE["p2"] = _build_phase2_fp8()
    return _CACHE["p2"]


def _phase2_fp16(ca):
    key = f"p2f16_{ca}"
    if key not in _CACHE:
        _CACHE[key] = _build_phase2_fp16(ca)
    return _CACHE[key]


def _host_consts():
    if "consts" not in _CACHE:
        kk = np.arange(P)[:, None]
        qq = np.arange(P)[None, :]
        tri = (qq >= kk).astype(np.float16)
        ident = np.eye(P, dtype=np.float16)
        _CACHE["consts"] = (tri, ident)
    return _CACHE["consts"]


def _q8(a):
    return a.astype(E4)


def kernel(x, Wq, Wk, Wv, Wo, bo):
    x = np.asarray(x, dtype=np.float32)
    Wq = np.asarray(Wq, dtype=np.float32)
    Wk = np.asarray(Wk, dtype=np.float32)
    Wv = np.asarray(Wv, dtype=np.float32)
    Wo = np.asarray(Wo, dtype=np.float32)
    bo = np.asarray(bo, dtype=np.float32)

    tri, ident = _host_consts()
    xt = np.ascontiguousarray(x.reshape(BT, C).T)
    x8 = _q8(xt)
    xl = _q8(16.0 * (xt - x8.astype(np.float32)))

    def _tile_w(a):                                 # [C, PD] -> [P, KS, PD]
        return np.ascontiguousarray(a.reshape(KS, P, PD).transpose(1, 0, 2))

    def wsplit(W, rs):
        wt = np.ascontiguousarray(W[rs].T)          # [C, PD] fp32
        w64 = _q8(64.0 * wt)
        wl = _q8(64.0 * wt - w64.astype(np.float32))
        w4 = _q8(4.0 * wt)
        return w64, wl, w4

    # host-peeled projections for chunks (0,0) and (1,0), matching the
    # device chain math: q64 = (w64+wl)^T x8 + w4^T xl (operands fp8)
    PEEL = (0, T)                                    # BT column starts

    def _peel(w64, wl, w4, three_term, t0):
        x8f = x8[:, t0 : t0 + CHUNK].astype(np.float32)
        acc = (w64.astype(np.float32) + wl.astype(np.float32)).T @ x8f
        if three_term:
            xlf = xl[:, t0 : t0 + CHUNK].astype(np.float32)
            acc += w4.astype(np.float32).T @ xlf
        return acc                                   # [PD, CHUNK], 64-scaled

    in_maps = []
    for c in range(NC):
        rs = slice(PD * c, PD * (c + 1))
        wq = wsplit(Wq, rs)
        wk = wsplit(Wk, rs)
        wv = wsplit(Wv, rs)
        im = {"x8": x8, "xl": xl,
              "wall": np.ascontiguousarray(np.stack(
                  [_tile_w(a) for a in (wq[0], wq[1], wq[2], wk[0], wk[1],
                                        wv[0], wv[1], wv[2])], axis=1)),
              "tri2": np.ascontiguousarray(np.stack([tri, ident], axis=1))}
        for i, t0 in enumerate(PEEL):
            q64 = _peel(*wq, False, t0)
            k64 = _peel(*wk, False, t0)
            v64 = _peel(*wv, True, t0)
            im[f"qp{i}"] = np.ascontiguousarray(_q8(q64))
            khi = _q8(k64)
            klo = _q8(k64 - khi.astype(np.float32))
            im[f"kp{i}"] = np.ascontiguousarray(np.stack([khi, klo], axis=1))
            v16 = v64.astype(np.float16)             # [PD, CHUNK] 64-scaled
            v0 = np.full((P, TPC, HPC, 2 * D), 64.0, dtype=np.float16)
            # v0[p, t, h, d] = v16[h*64+d, t*128+p]
            v0[:, :, :, 0:D] = v16.reshape(HPC, D, TPC, P).transpose(3, 2, 0, 1)
            im[f"v{i}"] = v0
        in_maps.append(im)
    res1 = bass_utils.run_bass_kernel_spmd(_phase1(), in_maps, core_ids=list(range(NC)))

    ctx = np.zeros((C, BT), dtype=np.float32)
    for c in range(NC):
        ctx[PD * c : PD * (c + 1)] = res1.results[c]["ctxt"].astype(np.float32)

    if not bo.any():
        chi = _q8(4.0 * ctx)
        clo = _q8(4.0 * ctx - chi.astype(np.float32))
        wot = np.ascontiguousarray(Wo.T)
        wo64 = _q8(64.0 * wot)
        wol = _q8(64.0 * wot - wo64.astype(np.float32))
        in_maps2 = [
            {"chi": np.ascontiguousarray(chi[:, ROWS2 * c : ROWS2 * (c + 1)]),
             "clo": np.ascontiguousarray(clo[:, ROWS2 * c : ROWS2 * (c + 1)]),
             "wo64": wo64, "wol": wol}
            for c in range(NC)
        ]
        res2 = bass_utils.run_bass_kernel_spmd(
            _phase2(), in_maps2, core_ids=list(range(NC))
        )
        out = np.concatenate(
            [res2.results[c]["o"] for c in range(NC)], axis=0
        ).astype(np.float32) * (1.0 / 256.0)
    else:
        ca = CA
        cta16 = np.zeros((ca, BT), dtype=np.float16)
        cta16[:C] = ctx.astype(np.float16)
        cta16[C, :] = 1.0
        woa = np.zeros((ca, C), dtype=np.float32)
        woa[:C] = Wo.T
        woa[C] = bo
        woa16 = woa.astype(np.float16)
        in_maps2 = [
            {"cta": np.ascontiguousarray(cta16[:, ROWS2 * c : ROWS2 * (c + 1)]),
             "woa": woa16}
            for c in range(NC)
        ]
        res2 = bass_utils.run_bass_kernel_spmd(
            _phase2_fp16(ca), in_maps2, core_ids=list(range(NC))
        )
        out = np.concatenate(
            [res2.results[c]["o"] for c in range(NC)], axis=0
        ).astype(np.float32)
    return out.reshape(B, T, C)


# revision 3
# speedup vs baseline: 1.0024x; 1.0024x over previous
"""Causal self-attention (B=2, T=2048, C=1024, H=16, d=64) on 8 Trainium2 NeuronCores.

Strategy (tensor-parallel over heads, two SPMD launches), fp8-DoubleRow edition:
  All heavy matmuls run as fp8e4 (e4m3) DoubleRow pairs (0.5 PE cycles per
  output column, 256-deep contraction per instruction).  e4m3 has a 2^-6
  minimum normal, so every tensor is pre-scaled by a power of two on the
  host to sit in the normal range, and the scales are folded into the exp
  activation scale / softmax-denominator ones-columns / final host unscale.
  Precision-critical paths (v, ctx, Wo) use hi+lo residual splits (effective
  ~11 mantissa bits, better than fp16); q/k tolerate single fp8 (3.6%).
  Measured end-to-end rel err of this scheme vs the fp32 reference: ~8e-3
  (gate is 2e-2).

  Launch 1 (head-parallel): core c owns heads {2c, 2c+1} (128 proj dims).
    Projections from host-split x8/xl and weight hi/lo tensors:
      q64 = (Wq64+Wql)^T x8 + Wq4^T xl      (12 DoubleRow matmuls / chunk)
      k64 = (Wk64+Wkl)^T x8                 (8)
      v64 = (Wv64+Wvl)^T x8 + Wv4^T xl      (12)
    Scores per (batch, head): DoubleRow with lhsT = (khi, klo) residual pair
    and rhs = (q8, q8): sc = (khi+klo)^T q8 at scale 64*64; exp applies
    0.125/4096.  Causal masking and the folded softmax denominator (ones
    columns, value 64.0 so the 64-scaled v cancels) are as before; AV runs
    in fp16.  Output ctxT [128, 4096] fp16, unscaled.
  Launch 2 (token-parallel, fp8 when bo==0): out*256 = (Wo64+Wol)^T chi
    + Wo64^T clo4 with chi = fp8(4 ctx), clo4 = fp8(4(ctx - chi/4)); host
    divides by 256.  Falls back to the fp16 kernel when bo != 0 (bias row).
"""
import sys

for _p in ("/opt/trn_rl_repo", "/root/.axon_site/_ro/trn_rl_repo"):
    if _p not in sys.path:
        sys.path.insert(0, _p)

import ml_dtypes
import numpy as np

import concourse.bass as bass  # noqa: F401  (registers bass types)
import concourse.tile as tile
from concourse import bacc, mybir
from concourse import bass_utils

B, T, C = 2, 2048, 1024
H, D = 16, 64
NC = 8
BT = B * T                       # 4096 tokens
HPC = H // NC                    # 2 heads per core
PD = HPC * D                     # 128 projection dims per core
P = 128
KS = C // P                      # 8 contraction subtiles
CHUNK = 512                      # token/query chunk
QCH = T // CHUNK                 # 4 query chunks per batch
TPC = CHUNK // P                 # 4 key tiles per chunk
KT = T // P                      # 16 key tiles per batch
CA = C + P                       # 1152 augmented contraction for fp16 phase 2
ROWS2 = BT // NC                 # 512 tokens per core in phase 2

F32 = mybir.dt.float32
F16 = mybir.dt.float16
F8 = mybir.dt.float8e4
E4 = ml_dtypes.float8_e4m3
EXP = mybir.ActivationFunctionType.Exp
DR = mybir.MatmulPerfMode.DoubleRow


def _build_phase1():
    nc = bacc.Bacc("TRN2", target_bir_lowering=False, debug=False, num_devices=NC)
    x8_ap = nc.dram_tensor("x8", [C, BT], F8, kind="ExternalInput").ap()
    xl_ap = nc.dram_tensor("xl", [C, BT], F8, kind="ExternalInput").ap()
    WNAMES = ("wq64", "wql", "wq4", "wk64", "wkl", "wv64", "wvl", "wv4")
    WIDX = {n: i for i, n in enumerate(WNAMES)}
    # all weights in one pre-tiled tensor [P, 8, KS, PD] (name-major so
    # group slices stay contiguous -- small runs pay a 2x DMA penalty)
    wa_ap = nc.dram_tensor("wall", [P, len(WNAMES), KS, PD], F8,
                           kind="ExternalInput").ap()
    t2_ap = nc.dram_tensor("tri2", [P, 2, P], F16, kind="ExternalInput").ap()
    # chunks (0,0) and (1,0) projections are host-peeled (prologue peeling)
    # so attention starts immediately while weights/x stream in
    peel_aps = {}
    for i in range(2):
        peel_aps[f"qp{i}"] = nc.dram_tensor(
            f"qp{i}", [P, CHUNK], F8, kind="ExternalInput").ap()
        peel_aps[f"kp{i}"] = nc.dram_tensor(
            f"kp{i}", [P, 2, CHUNK], F8, kind="ExternalInput").ap()
        peel_aps[f"v{i}"] = nc.dram_tensor(
            f"v{i}", [P, TPC, HPC, 2 * D], F16, kind="ExternalInput").ap()
    ct_ap = nc.dram_tensor("ctxt", [PD, BT], F16, kind="ExternalOutput").ap()

    x8_r = x8_ap.rearrange("(ks p) t -> p ks t", p=P)
    xl_r = xl_ap.rearrange("(ks p) t -> p ks t", p=P)

    with tile.TileContext(nc) as tc:
        with (
            tc.tile_pool(name="const", bufs=1) as const,
            tc.tile_pool(name="qkv", bufs=1) as qkv,
            tc.tile_pool(name="xt", bufs=3) as xtp,
            tc.tile_pool(name="vt", bufs=3) as vtp,
            tc.tile_pool(name="ep", bufs=16) as ep,
            tc.tile_pool(name="outp", bufs=3) as outp,
            tc.tile_pool(name="smallp", bufs=3) as smallp,
            tc.tile_pool(name="pp", bufs=2, space="PSUM") as pp,
            tc.tile_pool(name="scp", bufs=2, space="PSUM") as scp,
            tc.tile_pool(name="ctxp", bufs=2, space="PSUM") as ctxp,
        ):
            w_all = const.tile([P, len(WNAMES), KS, PD], F8, tag="wall")
            t2_sb = const.tile([P, 2, P], F16, tag="tri2")
            tri_sb = t2_sb[:, 0]
            id_sb = t2_sb[:, 1]

            # fp8 q8 tiles (broadcast to both DoubleRow slots via a stride-0
            # AP) and (khi,klo) residual pair tiles, per (batch, chunk)
            qpt = [[qkv.tile([P, CHUNK], F8, tag=f"qp{b}_{cc}", name=f"qp{b}_{cc}")
                    for cc in range(QCH)] for b in range(B)]
            kpt = [[qkv.tile([P, 2, CHUNK], F8, tag=f"kp{b}_{cc}", name=f"kp{b}_{cc}")
                    for cc in range(QCH)] for b in range(B)]
            # v in [token, dim] fp16 layout (64-scaled); columns D..2D are 64.0
            # so the AV matmul emits the 64-scaled softmax denominator and the
            # reciprocal-mul yields unscaled ctx.
            v_sb = [[qkv.tile([P, TPC, HPC, 2 * D], F16, tag=f"v{b}_{cc}",
                              name=f"v{b}_{cc}")
                     for cc in range(QCH)] for b in range(B)]

            def proj_bundles(b, cc):
                """(dma_thunk, [compute thunks]) for one chunk's projections.
                The dma thunk is emitted ~2 attention blocks ahead of use;
                compute thunks are interleaved into the previous chunk's
                attention block so the PE stream has independent work while
                ACT runs exps."""
                gsl = bass.ds(b * T + cc * CHUNK, CHUNK)
                st = {}

                def th_dma():
                    st["x8"] = xtp.tile([P, KS, CHUNK], F8, name="x8_t")
                    nc.sync.dma_start(st["x8"][:], x8_r[:, :, gsl])
                    st["xl"] = xtp.tile([P, KS, CHUNK], F8, name="xl_t")
                    nc.sync.dma_start(st["xl"][:], xl_r[:, :, gsl])

                def mk_chain(ps_name, terms):
                    # returns thunks each emitting `group` DR matmuls
                    mms = []
                    for wn, rk in terms:
                        for t in range(KS // 2):
                            mms.append((wn, rk, slice(2 * t, 2 * t + 2)))
                    n = len(mms)

                    def piece(lo, hi):
                        def th():
                            if lo == 0:
                                st[ps_name] = pp.tile(
                                    [P, CHUNK], F32, tag="pp", name=ps_name
                                )
                            ps = st[ps_name]
                            for i in range(lo, hi):
                                wn, rk, sl = mms[i]
                                nc.tensor.matmul(
                                    ps[:], w_all[:, WIDX[wn], sl, :],
                                    st[rk][:, sl],
                                    start=(i == 0), stop=(i == n - 1),
                                    perf_mode=DR,
                                )
                        return th

                    half = (n + 1) // 2
                    return [piece(0, half), piece(half, n)]

                def th_qcopy():
                    # step (0,1)'s copy rides the ACT engine: ACT has a gap
                    # right there (waiting for these projections), and this
                    # takes the copy off the serial DVE chain feeding sc(0,1)
                    if (b, cc) == (0, 1):
                        nc.scalar.copy(qpt[b][cc][:], st["ps_q"][:])
                    else:
                        nc.vector.tensor_copy(qpt[b][cc][:], st["ps_q"][:])

                def th_kcopy():
                    kp = kpt[b][cc]
                    nc.vector.tensor_copy(kp[:, 0], st["ps_k"][:])
                    nc.vector.tensor_sub(kp[:, 1], st["ps_k"][:], kp[:, 0])

                def th_vcopy():
                    st["vt"] = vtp.tile([P, CHUNK], F16, name="vt_t")
                    nc.vector.tensor_copy(st["vt"][:], st["ps_v"][:])

                def th_vtrans():
                    st["tr"] = pp.tile([P, CHUNK], F16, tag="pp", name="tr")
                    for j in range(TPC):
                        nc.tensor.transpose(
                            st["tr"][:, bass.ts(j, P)],
                            st["vt"][:, bass.ts(j, P)], id_sb[:],
                        )

                def th_vsb():
                    nc.vector.tensor_copy(
                        v_sb[b][cc][:, :, :, 0:D],
                        st["tr"][:].rearrange("p (j h d) -> p j h d", j=TPC, h=HPC),
                    )

                # q is 2-term (W-residual only): the requantization to fp8
                # for scores dominates its error anyway, and skipping the
                # xl chain unhooks q from the late-arriving xl DMA
                q1, q2 = mk_chain("ps_q", [("wq64", "x8"), ("wql", "x8")])
                k1, k2 = mk_chain("ps_k", [("wk64", "x8"), ("wkl", "x8")])
                v1, v2 = mk_chain("ps_v", [("wv64", "x8"), ("wvl", "x8"), ("wv4", "xl")])
                return th_dma, [k1, k2, th_kcopy, q1, q2, th_qcopy,
                                v1, v2, th_vcopy, th_vtrans, th_vsb]

            LOOK = 4  # kt-unit lookahead: sc/exp of kt+1 emitted before AV of kt

            def att_interleaved(b, ci, bundles, final=False):
                q0 = ci * CHUNK
                nkt = q0 // P + TPC
                nu = nkt
                ctx = [ctxp.tile([2 * D, CHUNK], F32, tag="ctx", name=f"ctx{ci}_{h}")
                       for h in range(HPC)]
                ets = {}

                def emit_sc(kt):
                    j = kt - q0 // P
                    c0 = 0 if j < 0 else P * j
                    kb = slice((kt % TPC) * P, (kt % TPC + 1) * P)
                    sc = scp.tile([P, 2, CHUNK], F32, tag="sc", name="sc")
                    for h in range(HPC):
                        dsl = slice(D * h, D * (h + 1))
                        qb = qpt[b][ci][dsl, c0:].rearrange(
                            "p (o n) -> p o n", o=1
                        ).broadcast_to([D, 2, CHUNK - c0])
                        nc.tensor.matmul(
                            sc[:, h, c0:],
                            kpt[b][kt // TPC][dsl, :, kb],
                            qb,
                            start=True, stop=True, perf_mode=DR,
                        )
                    e_t = ep.tile([P, 2, CHUNK], F16, name="e_t")
                    # one exp over both heads' live columns: halves the
                    # per-instruction ACT access-latency overhead
                    nc.scalar.activation(
                        e_t[:, :, c0:], sc[:, :, c0:], EXP, scale=0.125 / 4096.0
                    )
                    if j >= 0:
                        # early units' masks go to Pool so the DVE stream's
                        # head carries no exp-dependent waits (the next
                        # chunk's PSUM->SBUF copies must flow through first)
                        eng = nc.gpsimd if kt < 3 else nc.vector
                        for h in range(HPC):
                            eng.tensor_mul(
                                e_t[:, h, c0 : c0 + P], e_t[:, h, c0 : c0 + P],
                                tri_sb[:],
                            )
                    ets[kt] = (e_t, c0)

                def emit_av(kt):
                    e_t, c0 = ets.pop(kt)
                    for h in range(HPC):
                        nc.tensor.matmul(
                            ctx[h][:, c0:],
                            v_sb[b][kt // TPC][:, kt % TPC, h, 0 : 2 * D],
                            e_t[:, h, c0:],
                            start=(kt == 0), stop=(kt == nkt - 1),
                        )

                # bundles start after the first few sc units (so a bundle
                # stalled on a DMA never blocks this block's score stream)
                # and spread over a few units so copies still land early
                nb = len(bundles)
                total = nu + LOOK
                start = min(nu, 4)
                span = max(1, min(nu - 1, 9) - start)
                sched = {}
                for i in range(nb):
                    pos = min(total - 1, start + i * span // max(nb - 1, 1))
                    sched.setdefault(pos, []).append(bundles[i])
                for u in range(total):
                    if u < nu:
                        emit_sc(u)
                    for th in sched.get(u, ()):
                        th()
                    if u >= LOOK:
                        emit_av(u - LOOK)
                o_t = outp.tile([PD, CHUNK], F16, name="o_t")
                for h in range(HPC):
                    # (a single PSUM/PSUM divide is illegal: TensorTensor may
                    # read only one non-scalar input from PSUM)
                    r_t = smallp.tile([D, CHUNK], F32, tag="r", name="r_t")
                    nc.vector.reciprocal(r_t[:], ctx[h][D : 2 * D, :])
                    nc.vector.tensor_mul(
                        o_t[D * h : D * (h + 1), :], ctx[h][0:D, :], r_t[:]
                    )
                    if final:
                        # last block: DMA each head's half as soon as its
                        # reciprocal-mul lands, overlapping the other head's
                        nc.sync.dma_start(
                            ct_ap[D * h : D * (h + 1),
                                  b * T + q0 : b * T + q0 + CHUNK],
                            o_t[D * h : D * (h + 1), :],
                        )
                if not final:
                    # mid-stream output DMAs ride the Pool SWDGE queue so
                    # their semaphore waits never stall the SP SEQ (which
                    # must keep issuing x prefetches)
                    nc.gpsimd.dma_start(
                        ct_ap[:, b * T + q0 : b * T + q0 + CHUNK], o_t[:]
                    )

            steps = [(b, cc) for cc in range(QCH) for b in range(B)]
            dmas, comps = {}, {}
            for s in steps:
                if s[1] != 0:
                    dmas[s], comps[s] = proj_bundles(*s)
            # first-use-order loads on the sync queue: qp/kp of the two
            # peeled chunks unblock the first score matmuls, then tri/ident,
            # v0/v1 (first AVs), q/k weights, v weights.  (Not the scalar
            # queue: dma_starts there would stall the ACT SEQ and hence the
            # exp stream behind them.)
            nc.sync.dma_start(qpt[0][0][:], peel_aps["qp0"][:])
            nc.sync.dma_start(kpt[0][0][:], peel_aps["kp0"][:])
            nc.sync.dma_start(qpt[1][0][:], peel_aps["qp1"][:])
            nc.sync.dma_start(kpt[1][0][:], peel_aps["kp1"][:])
            nc.sync.dma_start(t2_sb[:], t2_ap[:])
            nc.sync.dma_start(v_sb[0][0][:], peel_aps["v0"][:])
            nc.sync.dma_start(v_sb[1][0][:], peel_aps["v1"][:])
            nc.sync.dma_start(w_all[:, 3:5], wa_ap[:, 3:5])
            dmas.pop(steps[2])()
            nc.sync.dma_start(w_all[:, 0:2], wa_ap[:, 0:2])
            nc.sync.dma_start(w_all[:, 5:], wa_ap[:, 5:])
            for i, (b, cc) in enumerate((b, cc) for b in range(B) for cc in range(QCH)):
                if cc == 0:
                    continue
                eng = nc.vector if i % 2 == 0 else nc.gpsimd
                eng.memset(v_sb[b][cc][:, :, :, D : 2 * D], 64.0)
            for j in range(1, len(steps) + 1):
                pre = dmas.pop(steps[j + 1], None) if j + 1 < len(steps) else None
                if pre is not None:
                    pre()
                att_interleaved(*steps[j - 1],
                                comps.pop(steps[j], []) if j < len(steps) else [],
                                final=(j == len(steps)))

    nc.compile()
    return nc


def _build_phase2_fp8():
    MT = ROWS2 // P              # 4 token tiles
    NT = C // CHUNK              # 2 output column tiles
    nc = bacc.Bacc("TRN2", target_bir_lowering=False, debug=False, num_devices=NC)
    chi_ap = nc.dram_tensor("chi", [C, ROWS2], F8, kind="ExternalInput").ap()
    clo_ap = nc.dram_tensor("clo", [C, ROWS2], F8, kind="ExternalInput").ap()
    wo64_ap = nc.dram_tensor("wo64", [C, C], F8, kind="ExternalInput").ap()
    wol_ap = nc.dram_tensor("wol", [C, C], F8, kind="ExternalInput").ap()
    o_ap = nc.dram_tensor("o", [ROWS2, C], F16, kind="ExternalOutput").ap()

    chi_r = chi_ap.rearrange("(ks p) t -> p ks t", p=P)
    clo_r = clo_ap.rearrange("(ks p) t -> p ks t", p=P)
    wo64_r = wo64_ap.rearrange("(ks p) n -> p ks n", p=P)
    wol_r = wol_ap.rearrange("(ks p) n -> p ks n", p=P)

    with tile.TileContext(nc) as tc:
        with (
            tc.tile_pool(name="ctp", bufs=8) as ctp,
            tc.tile_pool(name="wop", bufs=3) as wop,
            tc.tile_pool(name="outp", bufs=4) as outp,
            tc.tile_pool(name="ps", bufs=1, space="PSUM") as psp,
        ):
            NKP = KS // 2
            # n-outer: finish output-column half n before n+1, so the first
            # half's output copies/DMAs overlap the second half's compute.
            # chi/clo load once in halves on the sync HWDGE queue; the wo
            # tensors ride the Pool SWDGE queue (idle here) because every
            # dma_start costs 625ns on the single shared HWDGE unit.
            ps = [
                [psp.tile([P, CHUNK], F32, tag=f"ps{m}{n}", name=f"ps{m}{n}")
                 for m in range(MT)]
                for n in range(NT)
            ]
            warm = outp.tile([P, CHUNK], F16, name="warm")
            nc.gpsimd.memset(warm[:], 0.0)
            for _ in range(7):
                nc.tensor.matmul(ps[0][0][:], warm[:, 0:P], warm[:],
                                 start=True, stop=True)
            chi_t = ctp.tile([P, KS, ROWS2], F8, name="chi_t")
            clo_t = ctp.tile([P, KS, ROWS2], F8, name="clo_t")
            wo_tiles = {}
            for n in range(NT):
                nb = slice(n * CHUNK, (n + 1) * CHUNK)
                wo64_t = wop.tile([P, KS, CHUNK], F8, name="wo64_t")
                wol_t = wop.tile([P, KS, CHUNK], F8, name="wol_t")
                wo_tiles[n] = (wo64_t, wol_t)
                if n == 0:
                    nc.sync.dma_start(chi_t[:, 0:4], chi_r[:, 0:4])
                    nc.gpsimd.dma_start(wo64_t[:, 0:4], wo64_r[:, 0:4, nb])
                    nc.gpsimd.dma_start(wol_t[:, 0:4], wol_r[:, 0:4, nb])
                    nc.sync.dma_start(chi_t[:, 4:], chi_r[:, 4:])
                    nc.sync.dma_start(clo_t[:, 0:4], clo_r[:, 0:4])
                    nc.gpsimd.dma_start(wo64_t[:, 4:], wo64_r[:, 4:, nb])
                    nc.gpsimd.dma_start(wol_t[:, 4:], wol_r[:, 4:, nb])
                    nc.sync.dma_start(clo_t[:, 4:], clo_r[:, 4:])
                else:
                    nc.gpsimd.dma_start(wo64_t[:], wo64_r[:, :, nb])
                    nc.gpsimd.dma_start(wol_t[:], wol_r[:, :, nb])
            for n in range(NT):
                nb = slice(n * CHUNK, (n + 1) * CHUNK)
                wo64_t, wol_t = wo_tiles[n]
                for t in range(NKP):
                    sl = slice(2 * t, 2 * t + 2)
                    for m in range(MT):
                        mb = slice(m * P, (m + 1) * P)
                        nc.tensor.matmul(
                            ps[n][m][:], chi_t[:, sl, mb], wo64_t[:, sl],
                            start=(t == 0), stop=False, perf_mode=DR,
                        )
                        nc.tensor.matmul(
                            ps[n][m][:], chi_t[:, sl, mb], wol_t[:, sl],
                            start=False, stop=False, perf_mode=DR,
                        )
                        nc.tensor.matmul(
                            ps[n][m][:], clo_t[:, sl, mb], wo64_t[:, sl],
                            start=False, stop=(t == NKP - 1), perf_mode=DR,
                        )
                        if t == NKP - 1:
                            # drain this (m, n) while others still accumulate
                            o_sb = outp.tile([P, CHUNK], F16, name="o_sb")
                            if m % 2 == 0:
                                nc.vector.tensor_copy(o_sb[:], ps[n][m][:])
                            else:
                                nc.scalar.copy(o_sb[:], ps[n][m][:])
                            deng = nc.sync if m % 2 == 0 else nc.scalar
                            deng.dma_start(o_ap[bass.ts(m, P), nb], o_sb[:])

    nc.compile()
    return nc


def _build_phase2_fp16(ca):
    """fp16 fallback, used only when bo != 0 (bias-augmented contraction)."""
    KS2 = ca // P
    MT = ROWS2 // P
    NT = C // CHUNK
    nc = bacc.Bacc("TRN2", target_bir_lowering=False, debug=False, num_devices=NC)
    ct_ap = nc.dram_tensor("cta", [ca, ROWS2], F16, kind="ExternalInput").ap()
    wo_ap = nc.dram_tensor("woa", [ca, C], F16, kind="ExternalInput").ap()
    o_ap = nc.dram_tensor("o", [ROWS2, C], F16, kind="ExternalOutput").ap()

    ct_r = ct_ap.rearrange("(ks p) t -> p ks t", p=P)
    wo_r = wo_ap.rearrange("(ks p) n -> p ks n", p=P)

    with tile.TileContext(nc) as tc:
        with (
            tc.tile_pool(name="ctp", bufs=4) as ctp,
            tc.tile_pool(name="wop", bufs=4) as wop,
            tc.tile_pool(name="outp", bufs=4) as outp,
            tc.tile_pool(name="ps", bufs=1, space="PSUM") as psp,
        ):
            ps = [
                [psp.tile([P, CHUNK], F32, tag=f"ps{m}{n}", name=f"ps{m}{n}")
                 for n in range(NT)]
                for m in range(MT)
            ]
            for k in range(KS2):
                ct_t = ctp.tile([P, ROWS2], F16, name="ct_t")
                nc.sync.dma_start(ct_t[:], ct_r[:, k])
                wo_t = wop.tile([P, C], F16, name="wo_t")
                if k == 0:
                    nc.sync.dma_start(wo_t[:, 0:CHUNK], wo_r[:, k, 0:CHUNK])
                    nc.sync.dma_start(wo_t[:, CHUNK:], wo_r[:, k, CHUNK:])
                else:
                    nc.sync.dma_start(wo_t[:], wo_r[:, k])
                for m in range(MT):
                    for n in range(NT):
                        nc.tensor.matmul(
                            ps[m][n][:],
                            ct_t[:, bass.ts(m, P)],
                            wo_t[:, bass.ts(n, CHUNK)],
                            start=(k == 0), stop=(k == KS2 - 1),
                        )
            for m in range(MT):
                o_sb = outp.tile([P, C], F16, name="o_sb")
                nc.vector.tensor_copy(o_sb[:, 0:CHUNK], ps[m][0][:])
                nc.scalar.copy(o_sb[:, CHUNK:], ps[m][1][:])
                nc.sync.dma_start(o_ap[bass.ts(m, P), :], o_sb[:])

    nc.compile()
    return nc


_CACHE = {}


def _phase1():
    if "p1" not in _CACHE:
        _CACHE["p1"] = _build_phase1()
    return _CACHE["p1"]


def _phase2():
    if "p2" not in _CACHE:
        _CACHE["p2"] = _build_phase2_fp8()
    return _CACHE["p2"]


def _phase2_fp16(ca):
    key = f"p2f16_{ca}"
    if key not in _CACHE:
        _CACHE[key] = _build_phase2_fp16(ca)
    return _CACHE[key]


def _host_consts():
    if "consts" not in _CACHE:
        kk = np.arange(P)[:, None]
        qq = np.arange(P)[None, :]
        tri = (qq >= kk).astype(np.float16)
        ident = np.eye(P, dtype=np.float16)
        _CACHE["consts"] = (tri, ident)
    return _CACHE["consts"]


def _q8(a):
    return a.astype(E4)


def kernel(x, Wq, Wk, Wv, Wo, bo):
    x = np.asarray(x, dtype=np.float32)
    Wq = np.asarray(Wq, dtype=np.float32)
    Wk = np.asarray(Wk, dtype=np.float32)
    Wv = np.asarray(Wv, dtype=np.float32)
    Wo = np.asarray(Wo, dtype=np.float32)
    bo = np.asarray(bo, dtype=np.float32)

    tri, ident = _host_consts()
    xt = np.ascontiguousarray(x.reshape(BT, C).T)
    x8 = _q8(xt)
    xl = _q8(16.0 * (xt - x8.astype(np.float32)))

    def _tile_w(a):                                 # [C, PD] -> [P, KS, PD]
        return np.ascontiguousarray(a.reshape(KS, P, PD).transpose(1, 0, 2))

    def wsplit(W, rs):
        wt = np.ascontiguousarray(W[rs].T)          # [C, PD] fp32
        w64 = _q8(64.0 * wt)
        wl = _q8(64.0 * wt - w64.astype(np.float32))
        w4 = _q8(4.0 * wt)
        return w64, wl, w4

    # host-peeled projections for chunks (0,0) and (1,0), matching the
    # device chain math: q64 = (w64+wl)^T x8 + w4^T xl (operands fp8)
    PEEL = (0, T)                                    # BT column starts

    def _peel(w64, wl, w4, three_term, t0):
        x8f = x8[:, t0 : t0 + CHUNK].astype(np.float32)
        acc = (w64.astype(np.float32) + wl.astype(np.float32)).T @ x8f
        if three_term:
            xlf = xl[:, t0 : t0 + CHUNK].astype(np.float32)
            acc += w4.astype(np.float32).T @ xlf
        return acc                                   # [PD, CHUNK], 64-scaled

    in_maps = []
    for c in range(NC):
        rs = slice(PD * c, PD * (c + 1))
        wq = wsplit(Wq, rs)
        wk = wsplit(Wk, rs)
        wv = wsplit(Wv, rs)
        im = {"x8": x8, "xl": xl,
              "wall": np.ascontiguousarray(np.stack(
                  [_tile_w(a) for a in (wq[0], wq[1], wq[2], wk[0], wk[1],
                                        wv[0], wv[1], wv[2])], axis=1)),
              "tri2": np.ascontiguousarray(np.stack([tri, ident], axis=1))}
        for i, t0 in enumerate(PEEL):
            q64 = _peel(*wq, False, t0)
            k64 = _peel(*wk, False, t0)
            v64 = _peel(*wv, True, t0)
            im[f"qp{i}"] = np.ascontiguousarray(_q8(q64))
            khi = _q8(k64)
            klo = _q8(k64 - khi.astype(np.float32))
            im[f"kp{i}"] = np.ascontiguousarray(np.stack([khi, klo], axis=1))
            v16 = v64.astype(np.float16)             # [PD, CHUNK] 64-scaled
            v0 = np.full((P, TPC, HPC, 2 * D), 64.0, dtype=np.float16)
            # v0[p, t, h, d] = v16[h*64+d, t*128+p]
            v0[:, :, :, 0:D] = v16.reshape(HPC, D, TPC, P).transpose(3, 2, 0, 1)
            im[f"v{i}"] = v0
        in_maps.append(im)
    res1 = bass_utils.run_bass_kernel_spmd(_phase1(), in_maps, core_ids=list(range(NC)))

    ctx = np.zeros((C, BT), dtype=np.float32)
    for c in range(NC):
        ctx[PD * c : PD * (c + 1)] = res1.results[c]["ctxt"].astype(np.float32)

    if not bo.any():
        chi = _q8(4.0 * ctx)
        clo = _q8(4.0 * ctx - chi.astype(np.float32))
        wot = np.ascontiguousarray(Wo.T)
        wo64 = _q8(64.0 * wot)
        wol = _q8(64.0 * wot - wo64.astype(np.float32))
        in_maps2 = [
            {"chi": np.ascontiguousarray(chi[:, ROWS2 * c : ROWS2 * (c + 1)]),
             "clo": np.ascontiguousarray(clo[:, ROWS2 * c : ROWS2 * (c + 1)]),
             "wo64": wo64, "wol": wol}
            for c in range(NC)
        ]
        res2 = bass_utils.run_bass_kernel_spmd(
            _phase2(), in_maps2, core_ids=list(range(NC))
        )
        out = np.concatenate(
            [res2.results[c]["o"] for c in range(NC)], axis=0
        ).astype(np.float32) * (1.0 / 256.0)
    else:
        ca = CA
        cta16 = np.zeros((ca, BT), dtype=np.float16)
        cta16[:C] = ctx.astype(np.float16)
        cta16[C, :] = 1.0
        woa = np.zeros((ca, C), dtype=np.float32)
        woa[:C] = Wo.T
        woa[C] = bo
        woa16 = woa.astype(np.float16)
        in_maps2 = [
            {"cta": np.ascontiguousarray(cta16[:, ROWS2 * c : ROWS2 * (c + 1)]),
             "woa": woa16}
            for c in range(NC)
        ]
        res2 = bass_utils.run_bass_kernel_spmd(
            _phase2_fp16(ca), in_maps2, core_ids=list(range(NC))
        )
        out = np.concatenate(
            [res2.results[c]["o"] for c in range(NC)], axis=0
        ).astype(np.float32)
    return out.reshape(B, T, C)
